# revision 1
# baseline (speedup 1.0000x reference)
# Trainium2 Bass kernel for nn_AttentiveLinear.
#
# Math:  y[n,o] = sum_i x[n,i] * W[n,i,o] + b[n,o]
#        W[n,i,o] = (x @ Ww)[n, i*128+o] + bw[i*128+o]
#        b        = x @ Wb + bb
# Expanded:
#        y[n,o] = sum_i x[n,i] * T[n,i,o]  +  (x @ (Wb + BW))[n,o] + bb[o]
# with   T = x @ Ww (the 512MB intermediate, kept on-chip only) and
#        BW[i,o] = bw[i*128+o].
#
# Per-core plan (data-parallel over tokens, 1024 tokens/core):
#   pass 1: for each output o (=chunk c), matmul
#           Tc[i, tok] = Wq_c^T @ xT   (Wq_c = Ww columns for o=c, [j, i])
#           PSUM -> SBUF copies (DVE+ACT alternating) store T as bf16 in
#           token-major layout tb[i, tok, o] so each token's T_n[i, o] is a
#           contiguous 128x128 stationary operand.
#   pass 2: yT_psum[o, tok]  = lin^T @ xT  (linear part, one matmul)
#                            += per-token matmul T_n^T @ x_n (M=128 dense)
#           bias added during the PSUM->SBUF copy via per-partition scalar add.
# Host does all layout prep: x transpose/shard/cast, Ww column permutation,
# folding bw into the linear weight.

import numpy as np
import ml_dtypes

N_CORES = 8
IN_F = 128
OUT_F = 128
TOK_TOTAL = 8192
TOK = TOK_TOTAL // N_CORES  # 1024 tokens per core
G = 256                     # tokens per group
NG = TOK // G

_CACHE = {}
LAST_RESULT = None


def _build_program():
    import concourse.mybir as mybir
    import concourse.tile as tile
    from concourse import bacc

    dt = mybir.dt
    nc = bacc.Bacc(
        "TRN2", target_bir_lowering=False, debug=False, num_devices=N_CORES
    )

    xt_d = nc.dram_tensor("xt", [IN_F, TOK], dt.bfloat16, kind="ExternalInput")
    wq_d = nc.dram_tensor(
        "wq", [IN_F, IN_F * OUT_F], dt.bfloat16, kind="ExternalInput"
    )
    lin_d = nc.dram_tensor("lin", [IN_F, OUT_F], dt.bfloat16, kind="ExternalInput")
    bbc_d = nc.dram_tensor("bbc", [OUT_F, 1], dt.float32, kind="ExternalInput")
    yt_d = nc.dram_tensor("yt", [OUT_F, TOK], dt.float32, kind="ExternalOutput")

    with tile.TileContext(nc) as tc:
        with (
            tc.tile_pool(name="const", bufs=1) as const,
            tc.tile_pool(name="tbig", bufs=2) as tbigp,
            tc.tile_pool(name="ysb", bufs=2) as ysbp,
            tc.tile_pool(name="psch", bufs=3, space="PSUM") as psch,
            tc.tile_pool(name="psy", bufs=2, space="PSUM") as psyp,
        ):
            wq_s = const.tile([IN_F, IN_F * OUT_F], dt.bfloat16)
            for k in range(8):
                sl = slice(k * 2048, (k + 1) * 2048)
                nc.sync.dma_start(wq_s[:, sl], wq_d[:, sl])
            xt_s = const.tile([IN_F, TOK], dt.bfloat16)
            nc.sync.dma_start(xt_s[:], xt_d[:])
            lin_s = const.tile([IN_F, OUT_F], dt.bfloat16)
            nc.sync.dma_start(lin_s[:], lin_d[:])
            bbc_s = const.tile([OUT_F, 1], dt.float32)
            nc.sync.dma_start(bbc_s[:], bbc_d[:])

            for g in range(NG):
                gs = slice(g * G, (g + 1) * G)
                # tb[i, tok_in_group, o], bf16
                tb = tbigp.tile([IN_F, G, OUT_F], dt.bfloat16)

                # pass 1: produce T for this token group, 4 o-chunks per
                # PSUM tile so each PSUM->SBUF copy moves 4*G columns.
                for cq in range(OUT_F // 4):
                    ps = psch.tile([IN_F, 4, G], dt.float32)
                    for q in range(4):
                        c = cq * 4 + q
                        nc.tensor.matmul(
                            ps[:, q, :],
                            wq_s[:, c * IN_F : (c + 1) * IN_F],
                            xt_s[:, gs],
                            start=True,
                            stop=True,
                        )
                    out_ap = tb[:, :, cq * 4 : (cq + 1) * 4].transpose([0, 2, 1])
                    if cq % 2 == 0:
                        nc.vector.tensor_copy(out_ap, ps[:, :, :])
                    else:
                        nc.scalar.copy(out_ap, ps[:, :, :])

                # pass 2: linear part then per-token quadratic part, all
                # accumulating into one PSUM bank laid out [o, token].
                yp = psyp.tile([OUT_F, G], dt.float32)
                nc.tensor.matmul(
                    yp[:],
                    lin_s[:],
                    xt_s[:, gs],
                    start=True,
                    stop=False,
                    skip_group_check=True,
                )
                for t in range(G):
                    n = g * G + t
                    nc.tensor.matmul(
                        yp[:, t : t + 1],
                        tb[:, t, :],
                        xt_s[:, n : n + 1],
                        start=False,
                        stop=(t == G - 1),
                        skip_group_check=True,
                    )
                ys = ysbp.tile([OUT_F, G], dt.float32)
                nc.vector.tensor_scalar_add(ys[:], yp[:], bbc_s[:])
                nc.sync.dma_start(yt_d[:, gs], ys[:])

    nc.compile()
    return nc


def _host_prep(x, Wb, bb, Ww, bw):
    bf16 = ml_dtypes.bfloat16
    x = np.asarray(x, dtype=np.float32)
    Wb = np.asarray(Wb, dtype=np.float32)
    bb = np.asarray(bb, dtype=np.float32)
    Ww = np.asarray(Ww, dtype=np.float32)
    bw = np.asarray(bw, dtype=np.float32)

    xf = x.reshape(-1, IN_F)
    # Wq[j, o*128 + i] = Ww[j, i*128 + o]
    wq = np.ascontiguousarray(
        Ww.reshape(IN_F, IN_F, OUT_F).transpose(0, 2, 1)
    ).reshape(IN_F, IN_F * OUT_F).astype(bf16)
    lin = (Wb + bw.reshape(IN_F, OUT_F)).astype(bf16)
    bbc = np.ascontiguousarray(bb.reshape(OUT_F, 1))

    in_maps = []
    for c in range(N_CORES):
        sh = xf[c * TOK : (c + 1) * TOK]
        xt = np.ascontiguousarray(sh.T).astype(bf16)
        in_maps.append({"xt": xt, "wq": wq, "lin": lin, "bbc": bbc})
    return in_maps, x.shape


def kernel(x, Wb, bb, Ww, bw):
    global LAST_RESULT
    from concourse.bass_utils import run_bass_kernel_spmd

    in_maps, xshape = _host_prep(x, Wb, bb, Ww, bw)
    if "nc" not in _CACHE:
        _CACHE["nc"] = _build_program()
    nc = _CACHE["nc"]

    res = run_bass_kernel_spmd(nc, in_maps, core_ids=list(range(N_CORES)))
    LAST_RESULT = res
    y = np.concatenate(
        [res.results[c]["yt"].T for c in range(N_CORES)], axis=0
    )
    return np.ascontiguousarray(y.reshape(xshape[:-1] + (OUT_F,)), dtype=np.float32)


# revision 2
# speedup vs baseline: 3.0919x; 3.0919x over previous
# Trainium2 Bass kernel for nn_AttentiveLinear.
#
# Math:  y[n,o] = sum_i x[n,i] * W[n,i,o] + b[n,o]
#        W[n,i,o] = (x @ Ww)[n, i*128+o] + bw[i*128+o]
#        b        = x @ Wb + bb
# Expanded:
#        y[n,o] = sum_i x[n,i] * T[n,i,o]  +  (x @ (Wb + BW))[n,o] + bb[o]
# with   T = x @ Ww (the 512MB intermediate, kept on-chip only) and
#        BW[i,o] = bw[i*128+o].
#
# Per-core plan (data-parallel over tokens, 1024 tokens/core):
#   pass 1: for each output o (=chunk c), matmul
#           Tc[i, tok] = Wq_c^T @ xT   (Wq_c = Ww columns for o=c, [j, i])
#           PSUM -> SBUF copies (DVE+ACT alternating) store T as bf16 in
#           token-major layout tb[i, tok, o] so each token's T_n[i, o] is a
#           contiguous 128x128 stationary operand.
#   pass 2: yT_psum[o, tok]  = lin^T @ xT  (linear part, one matmul)
#                            += per-token matmul T_n^T @ x_n (M=128 dense)
#           bias added during the PSUM->SBUF copy via per-partition scalar add.
# Host does all layout prep: x transpose/shard/cast, Ww column permutation,
# folding bw into the linear weight.

import numpy as np
import ml_dtypes

N_CORES = 8
IN_F = 128
OUT_F = 128
TOK_TOTAL = 8192
TOK = TOK_TOTAL // N_CORES  # 1024 tokens per core
G = 256                     # tokens per group
NG = TOK // G

_CACHE = {}
LAST_RESULT = None


def _build_program():
    import concourse.mybir as mybir
    import concourse.tile as tile
    from concourse import bacc

    dt = mybir.dt
    nc = bacc.Bacc(
        "TRN2", target_bir_lowering=False, debug=False, num_devices=N_CORES
    )

    xt_d = nc.dram_tensor("xt", [IN_F, TOK], dt.bfloat16, kind="ExternalInput")
    wq_d = nc.dram_tensor(
        "wq", [IN_F, IN_F * OUT_F], dt.bfloat16, kind="ExternalInput"
    )
    lin_d = nc.dram_tensor("lin", [IN_F, OUT_F], dt.bfloat16, kind="ExternalInput")
    bbc_d = nc.dram_tensor("bbc", [OUT_F, 1], dt.float32, kind="ExternalInput")
    yt_d = nc.dram_tensor("yt", [OUT_F, TOK], dt.float32, kind="ExternalOutput")

    with tile.TileContext(nc) as tc:
        with (
            tc.tile_pool(name="const", bufs=1) as const,
            tc.tile_pool(name="tbig", bufs=2) as tbigp,
            tc.tile_pool(name="ysb", bufs=2) as ysbp,
            tc.tile_pool(name="psch", bufs=3, space="PSUM") as psch,
            tc.tile_pool(name="psy", bufs=2, space="PSUM") as psyp,
        ):
            wq_s = const.tile([IN_F, IN_F * OUT_F], dt.bfloat16)
            for k in range(8):
                sl = slice(k * 2048, (k + 1) * 2048)
                nc.sync.dma_start(wq_s[:, sl], wq_d[:, sl])
            xt_s = const.tile([IN_F, TOK], dt.bfloat16)
            nc.sync.dma_start(xt_s[:], xt_d[:])
            lin_s = const.tile([IN_F, OUT_F], dt.bfloat16)
            nc.sync.dma_start(lin_s[:], lin_d[:])
            bbc_s = const.tile([OUT_F, 1], dt.float32)
            nc.sync.dma_start(bbc_s[:], bbc_d[:])

            NQ = OUT_F // 4  # 32 quad-chunks per group

            def emit_pass2_tokens(g, yp, tb, t0, t1):
                # per-token matmuls accumulating y^T columns for group g
                for t in range(t0, t1):
                    n = g * G + t
                    nc.tensor.matmul(
                        yp[:, t : t + 1],
                        tb[:, t, :],
                        xt_s[:, n : n + 1],
                        start=False,
                        stop=(t == G - 1),
                        skip_group_check=True,
                    )

            def finish_group(g, yp):
                ys = ysbp.tile([OUT_F, G], dt.float32)
                nc.vector.tensor_scalar_add(ys[:], yp[:], bbc_s[:])
                nc.sync.dma_start(yt_d[:, g * G : (g + 1) * G], ys[:])

            prev = None  # (g, yp, tb) of previous group awaiting pass-2
            TPQ = G // NQ  # pass-2 tokens interleaved per quad-chunk
            for g in range(NG):
                gs = slice(g * G, (g + 1) * G)
                # tb[i, tok_in_group, o], bf16
                tb = tbigp.tile([IN_F, G, OUT_F], dt.bfloat16)

                # pass 1 for group g, with the previous group's pass-2
                # token-matmuls interleaved to keep the PE array warm.
                for cq in range(NQ):
                    ps = psch.tile([IN_F, 4, G], dt.float32)
                    for q in range(4):
                        c = cq * 4 + q
                        nc.tensor.matmul(
                            ps[:, q, :],
                            wq_s[:, c * IN_F : (c + 1) * IN_F],
                            xt_s[:, gs],
                            start=True,
                            stop=True,
                        )
                    # transposing copy: strided PSUM read, blocked SBUF write
                    in_ap = ps.transpose([0, 2, 1])          # [128, G, 4]
                    out_ap = tb[:, :, cq * 4 : (cq + 1) * 4]  # [128, G, 4]
                    if cq % 2 == 0:
                        nc.vector.tensor_copy(out_ap, in_ap)
                    else:
                        nc.scalar.copy(out_ap, in_ap)
                    if prev is not None:
                        emit_pass2_tokens(
                            prev[0], prev[1], prev[2], cq * TPQ, (cq + 1) * TPQ
                        )
                if prev is not None:
                    finish_group(prev[0], prev[1])

                # init this group's y^T PSUM bank with the linear part
                yp = psyp.tile([OUT_F, G], dt.float32)
                nc.tensor.matmul(
                    yp[:],
                    lin_s[:],
                    xt_s[:, gs],
                    start=True,
                    stop=False,
                    skip_group_check=True,
                )
                prev = (g, yp, tb)

            # drain the last group's pass-2
            emit_pass2_tokens(prev[0], prev[1], prev[2], 0, G)
            finish_group(prev[0], prev[1])

    nc.compile()
    return nc


def _host_prep(x, Wb, bb, Ww, bw):
    bf16 = ml_dtypes.bfloat16
    x = np.asarray(x, dtype=np.float32)
    Wb = np.asarray(Wb, dtype=np.float32)
    bb = np.asarray(bb, dtype=np.float32)
    Ww = np.asarray(Ww, dtype=np.float32)
    bw = np.asarray(bw, dtype=np.float32)

    xf = x.reshape(-1, IN_F)
    # Wq[j, o*128 + i] = Ww[j, i*128 + o]
    wq = np.ascontiguousarray(
        Ww.reshape(IN_F, IN_F, OUT_F).transpose(0, 2, 1)
    ).reshape(IN_F, IN_F * OUT_F).astype(bf16)
    lin = (Wb + bw.reshape(IN_F, OUT_F)).astype(bf16)
    bbc = np.ascontiguousarray(bb.reshape(OUT_F, 1))

    in_maps = []
    for c in range(N_CORES):
        sh = xf[c * TOK : (c + 1) * TOK]
        xt = np.ascontiguousarray(sh.T).astype(bf16)
        in_maps.append({"xt": xt, "wq": wq, "lin": lin, "bbc": bbc})
    return in_maps, x.shape


def kernel(x, Wb, bb, Ww, bw):
    global LAST_RESULT
    from concourse.bass_utils import run_bass_kernel_spmd

    in_maps, xshape = _host_prep(x, Wb, bb, Ww, bw)
    if "nc" not in _CACHE:
        _CACHE["nc"] = _build_program()
    nc = _CACHE["nc"]

    res = run_bass_kernel_spmd(nc, in_maps, core_ids=list(range(N_CORES)))
    LAST_RESULT = res
    y = np.concatenate(
        [res.results[c]["yt"].T for c in range(N_CORES)], axis=0
    )
    return np.ascontiguousarray(y.reshape(xshape[:-1] + (OUT_F,)), dtype=np.float32)


# revision 5
# speedup vs baseline: 3.2610x; 1.0547x over previous
# Trainium2 Bass kernel for nn_AttentiveLinear.
#
# Math:  y[n,o] = sum_i x[n,i] * W[n,i,o] + b[n,o]
#        W[n,i,o] = (x @ Ww)[n, i*128+o] + bw[i*128+o]
#        b        = x @ Wb + bb
# Expanded:
#        y[n,o] = sum_i x[n,i] * T[n,i,o]  +  (x @ (Wb + BW))[n,o] + bb[o]
# with   T = x @ Ww (the 512MB intermediate, kept on-chip only) and
#        BW[i,o] = bw[i*128+o].
#
# Per-core plan (data-parallel over tokens, 1024 tokens/core):
#   pass 1: for each output o (=chunk c), matmul
#           Tc[i, tok] = Wq_c^T @ xT   (Wq_c = Ww columns for o=c, [j, i])
#           PSUM -> SBUF copies (DVE+ACT alternating) store T as bf16 in
#           token-major layout tb[i, tok, o] so each token's T_n[i, o] is a
#           contiguous 128x128 stationary operand.
#   pass 2: yT_psum[o, tok]  = lin^T @ xT  (linear part, one matmul)
#                            += per-token matmul T_n^T @ x_n (M=128 dense)
#           bias added during the PSUM->SBUF copy via per-partition scalar add.
# Host does all layout prep: x transpose/shard/cast, Ww column permutation,
# folding bw into the linear weight.

import numpy as np
import ml_dtypes

N_CORES = 8
IN_F = 128
OUT_F = 128
TOK_TOTAL = 8192
TOK = TOK_TOTAL // N_CORES  # 1024 tokens per core
G = 256                     # tokens per group
NG = TOK // G

_CACHE = {}
LAST_RESULT = None


def _build_program():
    import concourse.mybir as mybir
    import concourse.tile as tile
    from concourse import bacc

    dt = mybir.dt
    nc = bacc.Bacc(
        "TRN2", target_bir_lowering=False, debug=False, num_devices=N_CORES
    )

    xt_d = nc.dram_tensor("xt", [IN_F, TOK], dt.bfloat16, kind="ExternalInput")
    wq_d = nc.dram_tensor(
        "wq", [IN_F, IN_F * OUT_F], dt.bfloat16, kind="ExternalInput"
    )
    lin_d = nc.dram_tensor("lin", [IN_F, OUT_F], dt.bfloat16, kind="ExternalInput")
    bbc_d = nc.dram_tensor("bbc", [OUT_F, 1], dt.float32, kind="ExternalInput")
    yt_d = nc.dram_tensor("yt", [OUT_F, TOK], dt.float32, kind="ExternalOutput")

    with tile.TileContext(nc) as tc:
        with (
            tc.tile_pool(name="const", bufs=1) as const,
            tc.tile_pool(name="tbig", bufs=2) as tbigp,
            tc.tile_pool(name="ysb", bufs=2) as ysbp,
            tc.tile_pool(name="psch", bufs=3, space="PSUM") as psch,
            tc.tile_pool(name="psy", bufs=2, space="PSUM") as psyp,
        ):
            xt_s = const.tile([IN_F, TOK], dt.bfloat16)
            nc.sync.dma_start(xt_s[:], xt_d[:])
            wq_s = const.tile([IN_F, IN_F * OUT_F], dt.bfloat16)
            for k in range(32):
                sl = slice(k * 512, (k + 1) * 512)
                nc.sync.dma_start(wq_s[:, sl], wq_d[:, sl])
            lin_s = const.tile([IN_F, OUT_F], dt.bfloat16)
            nc.sync.dma_start(lin_s[:], lin_d[:])
            bbc_s = const.tile([OUT_F, 1], dt.float32)
            nc.sync.dma_start(bbc_s[:], bbc_d[:])

            NQ = OUT_F // 4  # 32 quad-chunks per group

            def emit_pass2_tokens(g, yp, tb, t0, t1):
                # per-token matmuls accumulating y^T columns for group g
                for t in range(t0, t1):
                    n = g * G + t
                    nc.tensor.matmul(
                        yp[:, t : t + 1],
                        tb[:, t, :],
                        xt_s[:, n : n + 1],
                        start=False,
                        stop=(t == G - 1),
                        skip_group_check=True,
                    )

            def finish_group(g, yp):
                ys = ysbp.tile([OUT_F, G], dt.float32)
                nc.vector.tensor_scalar_add(ys[:], yp[:], bbc_s[:])
                nc.sync.dma_start(yt_d[:, g * G : (g + 1) * G], ys[:])

            prev = None  # (g, yp, tb) of previous group awaiting pass-2
            # Interleave the previous group's pass-2 starting at quad 4 so
            # the PE has chunk work queued while that group's final copies
            # drain (every token-matmul needs all 32 copies done).
            START_Q = 4
            bounds = np.linspace(0, G, NQ - START_Q + 1).astype(int)
            for g in range(NG):
                gs = slice(g * G, (g + 1) * G)
                # tb[i, tok_in_group, o], bf16
                tb = tbigp.tile([IN_F, G, OUT_F], dt.bfloat16)

                # pass 1 for group g, with the previous group's pass-2
                # token-matmuls interleaved to keep the PE array warm.
                for cq in range(NQ):
                    ps = psch.tile([IN_F, 4, G], dt.float32)
                    for q in range(4):
                        c = cq * 4 + q
                        nc.tensor.matmul(
                            ps[:, q, :],
                            wq_s[:, c * IN_F : (c + 1) * IN_F],
                            xt_s[:, gs],
                            start=True,
                            stop=True,
                        )
                    # transposing copy: strided PSUM read, blocked SBUF write
                    in_ap = ps.transpose([0, 2, 1])          # [128, G, 4]
                    out_ap = tb[:, :, cq * 4 : (cq + 1) * 4]  # [128, G, 4]
                    if cq % 2 == 0:
                        nc.vector.tensor_copy(out_ap, in_ap)
                    else:
                        nc.scalar.copy(out_ap, in_ap)
                    if prev is not None and cq >= START_Q:
                        emit_pass2_tokens(
                            prev[0],
                            prev[1],
                            prev[2],
                            int(bounds[cq - START_Q]),
                            int(bounds[cq - START_Q + 1]),
                        )
                if prev is not None:
                    finish_group(prev[0], prev[1])

                # init this group's y^T PSUM bank with the linear part
                yp = psyp.tile([OUT_F, G], dt.float32)
                nc.tensor.matmul(
                    yp[:],
                    lin_s[:],
                    xt_s[:, gs],
                    start=True,
                    stop=False,
                    skip_group_check=True,
                )
                prev = (g, yp, tb)

            # drain the last group's pass-2
            emit_pass2_tokens(prev[0], prev[1], prev[2], 0, G)
            finish_group(prev[0], prev[1])

    nc.compile()
    return nc


def _host_prep(x, Wb, bb, Ww, bw):
    bf16 = ml_dtypes.bfloat16
    x = np.asarray(x, dtype=np.float32)
    Wb = np.asarray(Wb, dtype=np.float32)
    bb = np.asarray(bb, dtype=np.float32)
    Ww = np.asarray(Ww, dtype=np.float32)
    bw = np.asarray(bw, dtype=np.float32)

    xf = x.reshape(-1, IN_F)
    # Wq[j, o*128 + i] = Ww[j, i*128 + o]
    wq = np.ascontiguousarray(
        Ww.reshape(IN_F, IN_F, OUT_F).transpose(0, 2, 1)
    ).reshape(IN_F, IN_F * OUT_F).astype(bf16)
    lin = (Wb + bw.reshape(IN_F, OUT_F)).astype(bf16)
    bbc = np.ascontiguousarray(bb.reshape(OUT_F, 1))

    in_maps = []
    for c in range(N_CORES):
        sh = xf[c * TOK : (c + 1) * TOK]
        xt = np.ascontiguousarray(sh.T).astype(bf16)
        in_maps.append({"xt": xt, "wq": wq, "lin": lin, "bbc": bbc})
    return in_maps, x.shape


def kernel(x, Wb, bb, Ww, bw):
    global LAST_RESULT
    from concourse.bass_utils import run_bass_kernel_spmd

    in_maps, xshape = _host_prep(x, Wb, bb, Ww, bw)
    if "nc" not in _CACHE:
        _CACHE["nc"] = _build_program()
    nc = _CACHE["nc"]

    res = run_bass_kernel_spmd(nc, in_maps, core_ids=list(range(N_CORES)))
    LAST_RESULT = res
    y = np.concatenate(
        [res.results[c]["yt"].T for c in range(N_CORES)], axis=0
    )
    return np.ascontiguousarray(y.reshape(xshape[:-1] + (OUT_F,)), dtype=np.float32)


# revision 14
# speedup vs baseline: 3.8828x; 1.1907x over previous
# Trainium2 Bass kernel for nn_AttentiveLinear.
#
# Math:  y[n,o] = sum_i x[n,i] * W[n,i,o] + b[n,o]
#        W[n,i,o] = (x @ Ww)[n, i*128+o] + bw[i*128+o]
#        b        = x @ Wb + bb
# Expanded:
#        y[n,o] = sum_i x[n,i] * T[n,i,o]  +  (x @ (Wb + BW))[n,o] + bb[o]
# with   T = x @ Ww (the 512MB intermediate, kept on-chip only) and
#        BW[i,o] = bw[i*128+o].
#
# Per-core plan (data-parallel over tokens, 1024 tokens/core):
#   pass 1: for each output o (=chunk c), matmul
#           Tc[i, tok] = Wq_c^T @ xT   (Wq_c = Ww columns for o=c, [j, i])
#           PSUM -> SBUF copies (DVE+ACT alternating) store T as bf16 in
#           token-major layout tb[i, tok, o] so each token's T_n[i, o] is a
#           contiguous 128x128 stationary operand.
#   pass 2: yT_psum[o, tok]  = lin^T @ xT  (linear part, one matmul)
#                            += per-token matmul T_n^T @ x_n (M=128 dense)
#           bias added during the PSUM->SBUF copy via per-partition scalar add.
# Host does all layout prep: x transpose/shard/cast, Ww column permutation,
# folding bw into the linear weight.

import numpy as np
import ml_dtypes

N_CORES = 8
IN_F = 128
OUT_F = 128
TOK_TOTAL = 8192
TOK = TOK_TOTAL // N_CORES  # 1024 tokens per core
G = 256                     # tokens per group
NG = TOK // G

_CACHE = {}
LAST_RESULT = None


def _build_program():
    import concourse.mybir as mybir
    import concourse.tile as tile
    from concourse import bacc

    from concourse.tile_rust import add_dep_helper

    dt = mybir.dt
    nc = bacc.Bacc(
        "TRN2", target_bir_lowering=False, debug=False, num_devices=N_CORES
    )

    xt_d = nc.dram_tensor("xt", [IN_F, TOK], dt.bfloat16, kind="ExternalInput")
    wq_d = nc.dram_tensor(
        "wq", [IN_F, IN_F * OUT_F], dt.bfloat16, kind="ExternalInput"
    )
    lin_d = nc.dram_tensor("lin", [IN_F, OUT_F], dt.bfloat16, kind="ExternalInput")
    bbc_d = nc.dram_tensor("bbc", [OUT_F, 1], dt.float32, kind="ExternalInput")
    yt_d = nc.dram_tensor("yt", [OUT_F, TOK], dt.float32, kind="ExternalOutput")

    with tile.TileContext(nc) as tc:
        with (
            tc.tile_pool(name="const", bufs=1) as const,
            tc.tile_pool(name="tbig", bufs=2) as tbigp,
            tc.tile_pool(name="ysb", bufs=2) as ysbp,
            tc.tile_pool(name="psch", bufs=3, space="PSUM") as psch,
            tc.tile_pool(name="psy", bufs=2, space="PSUM") as psyp,
        ):
            lin_s = const.tile([IN_F, OUT_F], dt.bfloat16)
            nc.sync.dma_start(lin_s[:], lin_d[:])
            bbc_s = const.tile([OUT_F, 1], dt.float32)
            nc.sync.dma_start(bbc_s[:], bbc_d[:])
            xt_s = const.tile([IN_F, TOK], dt.bfloat16)
            for g in range(NG):
                nc.gpsimd.dma_start(
                    xt_s[:, g * G : (g + 1) * G], xt_d[:, g * G : (g + 1) * G]
                )
            wq_s = const.tile([IN_F, IN_F * OUT_F], dt.bfloat16)
            dma_engines = [nc.sync, nc.gpsimd, nc.scalar]
            for k in range(32):
                sl = slice(k * 512, (k + 1) * 512)
                dma_engines[k % 3].dma_start(wq_s[:, sl], wq_d[:, sl])

            NQ = OUT_F // 4  # 32 quad-chunks per group

            def emit_pass2_tokens(g, yp, tb, t0, t1, after=None):
                # per-token matmuls accumulating y^T columns for group g
                last = None
                for t in range(t0, t1):
                    n = g * G + t
                    last = nc.tensor.matmul(
                        yp[:, t : t + 1],
                        tb[:, t, :],
                        xt_s[:, n : n + 1],
                        start=False,
                        stop=(t == G - 1),
                        skip_group_check=True,
                    )
                    if after is not None:
                        add_dep_helper(
                            last.ins,
                            after.ins,
                            sync=False,
                            reason="pass-2 batch after this quad's chunks",
                        )
                        after = None
                return last

            def finish_group(g, yp):
                ys = ysbp.tile([OUT_F, G], dt.float32)
                nc.vector.tensor_scalar_add(ys[:], yp[:], bbc_s[:])
                nc.sync.dma_start(yt_d[:, g * G : (g + 1) * G], ys[:])

            # HAM warmup: keep the PE array busy while the wq DMAs stream in,
            # so pass 1 starts at the warm (2.4 GHz) clock.
            wps = psch.tile([IN_F, 4, G], dt.float32, tag="ps")
            for w in range(20):
                nc.tensor.matmul(
                    wps[:, w % 4, :],
                    xt_s[:, 0:IN_F],
                    xt_s[:, 0:G],
                    start=True,
                    stop=True,
                )

            prev = None  # (g, yp, tb) of previous group awaiting pass-2
            # Interleave the previous group's pass-2 starting at quad 4 so
            # the PE has chunk work queued while that group's final copies
            # drain (every token-matmul needs all 32 copies done).
            START_Q = 4
            bounds = np.linspace(0, G, NQ - START_Q + 1).astype(int)
            last_tok = None  # forces the scheduler to keep the interleave
            for g in range(NG):
                gs = slice(g * G, (g + 1) * G)
                # tb[i, tok_in_group, o], bf16
                tb = tbigp.tile([IN_F, G, OUT_F], dt.bfloat16)

                # pass 1 for group g, with the previous group's pass-2
                # token-matmuls interleaved to keep the PE array warm.
                for cq in range(NQ):
                    ps = psch.tile([IN_F, 4, G], dt.float32)
                    last_chunk = None
                    for q in range(4):
                        c = cq * 4 + q
                        last_chunk = nc.tensor.matmul(
                            ps[:, q, :],
                            wq_s[:, c * IN_F : (c + 1) * IN_F],
                            xt_s[:, gs],
                            start=True,
                            stop=True,
                        )
                        if q == 0 and last_tok is not None:
                            add_dep_helper(
                                last_chunk.ins,
                                last_tok.ins,
                                sync=False,
                                reason="keep pass-2 interleaved with pass-1",
                            )
                            last_tok = None
                    # transposing copy: strided PSUM read, blocked SBUF write
                    in_ap = ps.transpose([0, 2, 1])          # [128, G, 4]
                    out_ap = tb[:, :, cq * 4 : (cq + 1) * 4]  # [128, G, 4]
                    if cq % 2 == 0:
                        nc.vector.tensor_copy(out_ap, in_ap)
                    else:
                        nc.scalar.copy(out_ap, in_ap)
                    if prev is not None and cq >= START_Q:
                        last_tok = emit_pass2_tokens(
                            prev[0],
                            prev[1],
                            prev[2],
                            int(bounds[cq - START_Q]),
                            int(bounds[cq - START_Q + 1]),
                            after=last_chunk,
                        )
                if prev is not None:
                    finish_group(prev[0], prev[1])

                # init this group's y^T PSUM bank with the linear part
                yp = psyp.tile([OUT_F, G], dt.float32)
                nc.tensor.matmul(
                    yp[:],
                    lin_s[:],
                    xt_s[:, gs],
                    start=True,
                    stop=False,
                    skip_group_check=True,
                )
                prev = (g, yp, tb)

            # drain the last group's pass-2
            emit_pass2_tokens(prev[0], prev[1], prev[2], 0, G)
            finish_group(prev[0], prev[1])

    nc.compile()
    return nc


def _host_prep(x, Wb, bb, Ww, bw):
    bf16 = ml_dtypes.bfloat16
    x = np.asarray(x, dtype=np.float32)
    Wb = np.asarray(Wb, dtype=np.float32)
    bb = np.asarray(bb, dtype=np.float32)
    Ww = np.asarray(Ww, dtype=np.float32)
    bw = np.asarray(bw, dtype=np.float32)

    xf = x.reshape(-1, IN_F)
    # Wq[j, o*128 + i] = Ww[j, i*128 + o]
    wq = np.ascontiguousarray(
        Ww.reshape(IN_F, IN_F, OUT_F).transpose(0, 2, 1)
    ).reshape(IN_F, IN_F * OUT_F).astype(bf16)
    lin = (Wb + bw.reshape(IN_F, OUT_F)).astype(bf16)
    bbc = np.ascontiguousarray(bb.reshape(OUT_F, 1))

    in_maps = []
    for c in range(N_CORES):
        sh = xf[c * TOK : (c + 1) * TOK]
        xt = np.ascontiguousarray(sh.T).astype(bf16)
        in_maps.append({"xt": xt, "wq": wq, "lin": lin, "bbc": bbc})
    return in_maps, x.shape


def kernel(x, Wb, bb, Ww, bw):
    global LAST_RESULT
    from concourse.bass_utils import run_bass_kernel_spmd

    in_maps, xshape = _host_prep(x, Wb, bb, Ww, bw)
    if "nc" not in _CACHE:
        _CACHE["nc"] = _build_program()
    nc = _CACHE["nc"]

    res = run_bass_kernel_spmd(nc, in_maps, core_ids=list(range(N_CORES)))
    LAST_RESULT = res
    y = np.concatenate(
        [res.results[c]["yt"].T for c in range(N_CORES)], axis=0
    )
    return np.ascontiguousarray(y.reshape(xshape[:-1] + (OUT_F,)), dtype=np.float32)
